# revision 8
# baseline (speedup 1.0000x reference)
"""ForgetMult recurrence kernel for Trainium2 (Bass/Tile), 8-core SPMD.

h_t = f_t * x_t + (1 - f_t) * h_{t-1},  h_0 = 0
shapes: f, x, h = [seq=2048, batch=64, hidden=512] fp32

Strategy
--------
- Shard over batch: core k owns batches [8k, 8k+8) -> 4096 channels.
- fp16 I/O halves HBM traffic vs fp32 (48 MB/core: 32 in + 16 out; the
  memory roofline is ~134 us at 358 GB/s/core). Tolerance is 2e-2 and
  the end-to-end fp16 error is ~1e-3: the DVE scan keeps its
  recurrence state in fp32 internally regardless of operand dtype.
- The host lays data out channel-major (seq = SBUF free dim, so no
  device transposes) and packs TWO 128-channel subblocks per DRAM row
  group: fx row m*128+p = [f(m,j=0,p) | f(m,j=1,p) | x(m,j=0,p) |
  x(m,j=1,p)], 16 KB/row. Every device access is then a plain 2D
  contiguous slice -- no strided APs that would knock the DVE/ACT off
  their fast modes.
- 16 pipeline iterations/core: one fully-contiguous 2MB load, one ACT
  activation a = 1-f over 4096 cols, a [128,1] ACT zero at column
  2048 (the second subblock's first seq position), one DVE
  tensor_tensor b = f*x (2x fp16 mode), ONE 4096-col DVE scan
  covering both subblocks (the zeroed a column resets the state there:
  0*prev + b), one 1MB store.
- Why merged: the scan is the DVE floor (no fast perf modes; measured
  ~1.80 cyc/elem at FD=4096, its best rate -- 122 us/core busy) and
  every instruction adds semaphore/dispatch overhead on the
  near-saturated DVE+DMA overlap, so fewer+wider instructions win.
  GpSimd/PE/PSUM stay idle (Pool cannot scan, and Pool streaming
  serializes against a busy DVE on the shared SBUF port -- measured).
"""

import numpy as np

import concourse.bacc as bacc
import concourse.mybir as mybir
from concourse.tile import TileContext
from concourse.bass_utils import run_bass_kernel_spmd

SEQ, BATCH, HIDDEN = 2048, 64, 512
N_CORES = 8
B_PER_CORE = BATCH // N_CORES          # 8
CHANS = B_PER_CORE * HIDDEN            # 4096 channels per core
P = 128                                # SBUF partitions
MERGE = 2                              # subblocks per pipeline iteration
W = MERGE * SEQ                        # scan width per iteration (4096)
ROWS = CHANS // MERGE                  # 2048 DRAM rows per core


def _emit_program(nc, fx_d, h_d, reps, pre=None, post=None):
    """fx_d: [ROWS, 2*W] fp16 (row = f|f|x|x for a merge pair);
    h_d: [ROWS, W] fp16 (row = h|h)."""
    f16 = mybir.dt.float16
    Alu = mybir.AluOpType
    Act = mybir.ActivationFunctionType

    n_it = ROWS // P                   # 16 iterations
    AHEAD = 1                          # scan trails the TT by one iteration
    ST_LAG = 3                         # store g is EMITTED after load g+ST_LAG

    with (
        TileContext(nc) as tc,
        tc.tile_pool(name="const", bufs=1) as cpool,
        tc.tile_pool(name="io", bufs=3) as iopool,
        tc.tile_pool(name="work", bufs=3) as wpool,
        tc.tile_pool(name="hout", bufs=6) as hpool,
    ):
        if pre is not None:
            pre(nc, tc, cpool)

        if reps > 1:
            # dynamic repetition for timing: constant code size, any trip
            # count; each iteration recomputes the same (correct) output
            loop_ctx = tc.For_i(0, reps, 1)
            loop_ctx.__enter__()

        # Stores go on the SP ring, NOT the ACT ring: a store's wait (its
        # scan) must never sit in the ACT stream where the in-order ACT
        # sequencer would stall the next activation behind it -- that puts a
        # scan->ACT->scan round trip into every iteration (measured +30us).
        # On SP, the store is emitted ST_LAG loads late, so by the time SP
        # reaches it, its scan finished long ago and SP never blocks.
        def do_scan(aT, bT, r0):
            hT = hpool.tile([P, W], f16, tag="h")
            # one scan covers both subblocks; a[:, SEQ] == 0 resets the
            # state at the second subblock's first column
            nc.vector.tensor_tensor_scan(
                hT[:], aT[:], bT[:], 0.0, Alu.mult, Alu.add
            )
            return (hT, r0)

        stages = []   # (aT, bT, r0) awaiting their scan
        done = []     # (hT, r0) awaiting their store
        n_stored = 0
        for g in range(n_it):
            r0 = g * P
            fxT = iopool.tile([P, 2 * W], f16, tag="fx")
            nc.sync.dma_start(out=fxT[:], in_=fx_d[r0 : r0 + P, :])
            if g >= ST_LAG:
                hT, hr0 = done[n_stored]
                nc.sync.dma_start(out=h_d[hr0 : hr0 + P, :], in_=hT[:])
                n_stored += 1

            aT = wpool.tile([P, W], f16, tag="a")
            bT = wpool.tile([P, W], f16, tag="b")
            nc.scalar.activation(
                aT[:], fxT[:, 0:W], Act.Copy, bias=1.0, scale=-1.0
            )
            nc.scalar.mul(aT[:, SEQ : SEQ + 1], aT[:, SEQ : SEQ + 1], 0.0)
            nc.vector.tensor_tensor(
                bT[:], fxT[:, 0:W], fxT[:, W : 2 * W], Alu.mult
            )
            stages.append((aT, bT, r0))
            if g >= AHEAD:
                done.append(do_scan(*stages[g - AHEAD]))

        for s in stages[n_it - AHEAD :]:
            done.append(do_scan(*s))
        for hT, hr0 in done[n_stored:]:
            nc.sync.dma_start(out=h_d[hr0 : hr0 + P, :], in_=hT[:])

        if reps > 1:
            loop_ctx.__exit__(None, None, None)

        if post is not None:
            post(nc, tc, cpool)


def build_nc(reps=1):
    """Build the single-core Bass program (same NEFF runs SPMD on all cores)."""
    f16 = mybir.dt.float16
    nc = bacc.Bacc("TRN2", target_bir_lowering=False, debug=False)
    fx_d = nc.dram_tensor("fx", [ROWS, 2 * W], f16, kind="ExternalInput").ap()
    h_d = nc.dram_tensor("h", [ROWS, W], f16, kind="ExternalOutput").ap()
    _emit_program(nc, fx_d, h_d, reps)
    nc.finalize()
    return nc


def build_bench_nc(reps):
    """Timing variant: fx/h live in Internal DRAM scratch so external I/O is
    tiny (the axon per-call overhead scales with I/O bytes). The dummy shape
    depends on reps so compile caches can't alias variants. The dummy output
    reads a slice of h to keep the pipeline live."""
    f16 = mybir.dt.float16
    nc = bacc.Bacc("TRN2", target_bir_lowering=False, debug=False)
    cols = 140 + reps  # matches test.py bench maps
    d_in = nc.dram_tensor("dummy_in", [P, cols], f16, kind="ExternalInput").ap()
    d_out = nc.dram_tensor("dummy_out", [P, cols], f16, kind="ExternalOutput").ap()
    fx_d = nc.dram_tensor("fxs", [ROWS, 2 * W], f16, kind="Internal").ap()
    h_d = nc.dram_tensor("hs", [ROWS, W], f16, kind="Internal").ap()

    def pre(nc, tc, cpool):
        # fill the scratch input with benign constants (f=0.5, x=1.0)
        zfx = cpool.tile([P, 2 * W], f16, tag="bench_zfx")
        nc.vector.memset(zfx[:, 0:W], 0.5)
        nc.vector.memset(zfx[:, W : 2 * W], 1.0)
        for g in range(ROWS // P):
            nc.sync.dma_start(out=fx_d[g * P : (g + 1) * P, :], in_=zfx[:])

    def post(nc, tc, cpool):
        # h[p, t] = 1 - 0.5^(t+1) for t < SEQ; out = 1 + h-slice
        t_in = cpool.tile([P, cols], f16, tag="bench_in")
        t_h = cpool.tile([P, cols], f16, tag="bench_h")
        nc.sync.dma_start(out=t_in[:], in_=d_in[:])
        nc.sync.dma_start(out=t_h[:], in_=h_d[0:P, 0:cols])
        nc.vector.tensor_tensor(t_in[:], t_in[:], t_h[:], mybir.AluOpType.add)
        nc.sync.dma_start(out=d_out[:], in_=t_in[:])

    _emit_program(nc, fx_d, h_d, reps, pre=pre, post=post)
    nc.finalize()
    return nc


_NC_CACHE = {}


def _get_nc():
    if "nc" not in _NC_CACHE:
        _NC_CACHE["nc"] = build_nc()
    return _NC_CACHE["nc"]


def kernel(f, x):
    f = np.asarray(f, dtype=np.float32).reshape(SEQ, BATCH, HIDDEN)
    x = np.asarray(x, dtype=np.float32).reshape(SEQ, BATCH, HIDDEN)
    f16 = f.astype(np.float16)
    x16 = x.astype(np.float16)
    nc = _get_nc()
    in_maps = []
    for k in range(N_CORES):
        b0 = k * B_PER_CORE
        # [seq, 8, 512] -> channel-major [4096, seq]
        fc = (
            f16[:, b0 : b0 + B_PER_CORE, :].transpose(1, 2, 0).reshape(CHANS, SEQ)
        )
        xc = (
            x16[:, b0 : b0 + B_PER_CORE, :].transpose(1, 2, 0).reshape(CHANS, SEQ)
        )
        # merge-pack: row m*128+p = [f(m,0,p) f(m,1,p) x(m,0,p) x(m,1,p)]
        fcv = fc.reshape(ROWS // P, MERGE, P, SEQ).transpose(0, 2, 1, 3)
        xcv = xc.reshape(ROWS // P, MERGE, P, SEQ).transpose(0, 2, 1, 3)
        fx = np.empty((ROWS, 2 * W), np.float16)
        fx[:, 0:W] = fcv.reshape(ROWS, W)
        fx[:, W : 2 * W] = xcv.reshape(ROWS, W)
        in_maps.append({"fx": fx})
    res = run_bass_kernel_spmd(nc, in_maps, core_ids=list(range(N_CORES)))
    hs = []
    for r in res.results:
        # [ROWS, W] -> channel-major [CHANS, SEQ] -> [seq, 8, 512]
        hv = (
            r["h"]
            .reshape(ROWS // P, P, MERGE, SEQ)
            .transpose(0, 2, 1, 3)
            .reshape(CHANS, SEQ)
        )
        hs.append(hv.reshape(B_PER_CORE, HIDDEN, SEQ).transpose(2, 0, 1))
    return np.concatenate(hs, axis=1).astype(np.float32)
